# revision 43
# baseline (speedup 1.0000x reference)
"""Trainium2 Bass kernel for nn_Attention_40510131536197.

RoPE attention layer with gated adapter (Llama-adapter style), B=2, S=1024,
D=4096, H=32 heads, head_dim=128, adapter_len=64.

Distribution: tensor-parallel over heads across 8 NeuronCores. Each core owns
4 heads (a 512-row slice of Wq/Wk/Wv, its gate slice and adapter-KV slice)
plus a 512-column slice of Wo. The attention output (feature-major layout
[512, 2048] per core) is AllGather'd across cores in four 512-token chunks,
then each core computes its 512-feature output slice of the Wo projection.
The host concatenates + transposes the 8 slices into the full
[2, 1024, 4096] output.

Phase interleaving hides every collective behind TensorEngine work:
  proj(batch0) -> attention(batch0) + AG0/AG1 -> proj(batch1)
  -> [Wo(ch0, ch1) | attention(batch1) | AG2/AG3] -> Wo(ch2, ch3)

All matmuls run in bf16 (f32 PSUM accumulation). Layout choices keep the
contraction dim on SBUF partitions for every matmul so the kernel needs no
on-device transposes:
  - projections computed as out.T = W.T-shard (lhsT) x x.T (rhs)
  - scores computed transposed: S_T[k, q]; softmax denominators via
    ones-broadcast matmuls; the additive mask is folded in on the PE via an
    identity-matrix matmul on the diagonal 128x128 blocks only (causal
    structure skips sub-diagonal blocks entirely)
  - PV uses V tiles (token-major) as lhsT and S_T as rhs, accumulating
    kt-outer with column-suffix ranges, so the attention output lands
    directly in the feature-major layout the Wo matmul wants.
RoPE runs on Vector/Scalar engines in the feature-major layout using a
host-side even/odd row permutation of Wq/Wk (invariant under the q.k
contraction).
"""

import math
import numpy as np
import ml_dtypes

# ---------------------------------------------------------------- constants
B, S, D, H = 2, 1024, 4096, 32
HD = D // H            # 128 head dim
AL = 64                # adapter length
NCORES = 8
HPC = H // NCORES      # 4 heads per core
DSL = HPC * HD         # 512 per-core feature slice
T = B * S              # 2048 tokens
SCALE = 1.0 / math.sqrt(HD)
NKT = S // 128         # 8 key tiles per batch
NCH = 4                # AllGather chunks (512 tokens each)

BF16 = ml_dtypes.bfloat16

_nc_cache = None


# ------------------------------------------------------------------- device
def _build_nc():
    import concourse.bacc as bacc
    import concourse.tile as tile
    from concourse import mybir
    from contextlib import ExitStack

    F32 = mybir.dt.float32
    BF = mybir.dt.bfloat16
    EXP = mybir.ActivationFunctionType.Exp
    TANH = mybir.ActivationFunctionType.Tanh
    MUL = mybir.AluOpType.mult

    nc = bacc.Bacc("TRN2", target_bir_lowering=False, debug=False,
                   num_devices=NCORES)

    # ---- external I/O (per-core shards; host prepares layouts/dtypes)
    xT = nc.dram_tensor("xT", [D, T], BF, kind="ExternalInput")
    wqT = nc.dram_tensor("wqT", [D, DSL], BF, kind="ExternalInput")
    wkT = nc.dram_tensor("wkT", [D, DSL], BF, kind="ExternalInput")
    wvT = nc.dram_tensor("wvT", [D, DSL], BF, kind="ExternalInput")
    woT = nc.dram_tensor("woT", [D, DSL], BF, kind="ExternalInput")
    adT = nc.dram_tensor("adT", [D, AL], BF, kind="ExternalInput")
    mdT = nc.dram_tensor("mdT", [128, NKT * 128], BF, kind="ExternalInput")
    fcos = nc.dram_tensor("fcos", [128, S], BF, kind="ExternalInput")
    fsin = nc.dram_tensor("fsin", [128, S], BF, kind="ExternalInput")
    gate = nc.dram_tensor("gate", [1, HPC], F32, kind="ExternalInput")
    ident = nc.dram_tensor("ident", [128, 128], BF, kind="ExternalInput")
    out = nc.dram_tensor("out", [DSL, T], F32, kind="ExternalOutput")

    KD = D // 128  # 32 contraction strips

    with tile.TileContext(nc) as tc, ExitStack() as stack:
        # ---------------- constants / persistent SBUF
        const = stack.enter_context(tc.tile_pool(name="const", bufs=1))
        qkv_sb = stack.enter_context(tc.tile_pool(name="qkv_sb", bufs=1))
        misc_sb = stack.enter_context(tc.tile_pool(name="misc_sb", bufs=1))
        dram = stack.enter_context(tc.tile_pool(name="dram", bufs=1,
                                                space="DRAM"))

        fcos_sb = const.tile([128, S], BF, name="fcos_sb")
        nc.sync.dma_start(fcos_sb[:], fcos[:, :])
        fsin_sb = const.tile([128, S], BF, name="fsin_sb")
        nc.sync.dma_start(fsin_sb[:], fsin[:, :])
        md_sb = const.tile([128, NKT * 128], BF, name="md_sb")
        nc.sync.dma_start(md_sb[:], mdT[:, :])
        ones_bc = const.tile([128, 128], BF, name="ones_bc")
        nc.vector.memset(ones_bc[:], 1.0)
        ident_bf = const.tile([128, 128], BF, name="ident_bf")
        nc.sync.dma_start(ident_bf[:], ident[:, :])
        # tanh(gate), broadcast to all partitions via a DRAM bounce
        g_raw = const.tile([1, HPC], F32, name="g_raw")
        nc.sync.dma_start(g_raw[:], gate[:, :])
        tg_row = const.tile([1, HPC], F32, name="tg_row")
        nc.scalar.activation(tg_row[:], g_raw[:], TANH)
        tg_d = dram.tile([1, HPC], F32, name="tg_d")
        nc.sync.dma_start(tg_d[:], tg_row[:])
        tg128 = const.tile([128, HPC], F32, name="tg128")
        nc.sync.dma_start(tg128[:], tg_d[0:1, :].to_broadcast((128, HPC)))

        # persistent activations
        QT = [qkv_sb.tile([128, T], BF, name=f"QT{h}") for h in range(HPC)]
        KT = [qkv_sb.tile([128, T], BF, name=f"KT{h}") for h in range(HPC)]
        Vt = [qkv_sb.tile([128, DSL], BF, name=f"V{i}")
              for i in range(T // 128)]
        akT_sb = misc_sb.tile([128, HPC * AL], BF, name="akT_sb")
        av_sb = misc_sb.tile([128, DSL], BF, name="av_sb")  # use [:AL]

        # collective bounce buffers, one per (512-token chunk, head) so each
        # head's slice gathers as soon as its epilogue lands
        ag_in = [[dram.tile([HD, 512], BF, name=f"ag_in{ch}_{h}")
                  for h in range(HPC)] for ch in range(NCH)]
        ag_out = [[dram.tile([NCORES * HD, 512], BF, addr_space="Shared",
                             name=f"ag_out{ch}_{h}") for h in range(HPC)]
                  for ch in range(NCH)]

        # ---------------------------------------------------------- helpers
        def emit_proj_half(th):
            """Projections (V, Q+RoPE, K+RoPE, adapter KV on half 0) for one
            1024-token half (== batch)."""
            t0 = th * S
            with tc.tile_pool(name=f"xp{th}", bufs=8) as xp, \
                 tc.tile_pool(name=f"wp{th}", bufs=4) as wp, \
                 tc.tile_pool(name=f"pps{th}", bufs=4, space="PSUM") as pps, \
                 tc.tile_pool(name=f"adps{th}", bufs=1, space="PSUM") as adps, \
                 tc.tile_pool(name=f"rp{th}", bufs=3) as rp:

                def rope_evac(ps, dst_ap, scol):
                    """RoPE: out = v*C + rot(v)*S' (rows are [evens; odds]).
                    The rotation's half-swap runs as two 1-input ACT copies
                    (2-input SBUF ops must share base partitions)."""
                    vbf = rp.tile([128, 512], BF, name="vbf", tag="rv")
                    nc.vector.tensor_copy(vbf[:], ps[:])
                    vrot = rp.tile([128, 512], BF, name="vrot", tag="rr")
                    nc.vector.tensor_copy(vrot[0:64, :], ps[64:128, :])
                    nc.vector.tensor_copy(vrot[64:128, :], ps[0:64, :])
                    tcv = rp.tile([128, 512], BF, name="tcv", tag="rc")
                    nc.vector.tensor_mul(tcv[:], vbf[:],
                                         fcos_sb[:, scol:scol + 512])
                    tsv = rp.tile([128, 512], BF, name="tsv", tag="rs")
                    nc.vector.tensor_mul(tsv[:], vrot[:],
                                         fsin_sb[:, scol:scol + 512])
                    nc.vector.tensor_add(dst_ap, tcv[:], tsv[:])

                if th == 0:
                    ad_g = misc_sb.tile([128, KD, AL], BF, name="ad_g")
                    nc.sync.dma_start(
                        ad_g[:], adT.rearrange("(k p) a -> p k a", p=128))
                    ad_strip = [ad_g[:, k, :] for k in range(KD)]
                    ak_ps = adps.tile([128, HPC * AL], F32, name="ak_ps",
                                      tag="adk")
                    av_ps = adps.tile([128, DSL], F32, name="av_ps",
                                      tag="adv")

                # interleave x / wv group loads so matmuls start early
                x_strip = []
                wv_s = []
                for g in range(KD // 4):
                    xg = xp.tile([128, 4, S], BF, name=f"x{th}_{g}",
                                 tag="xstrip")
                    nc.sync.dma_start(
                        xg[:],
                        xT[512 * g:512 * (g + 1),
                           t0:t0 + S].rearrange("(k p) t -> p k t", p=128))
                    x_strip.extend(xg[:, i, :] for i in range(4))
                    if g % 2 == 0:
                        gw = g // 2
                        wg = wp.tile([128, 8, DSL], BF, name=f"wv{th}_{gw}",
                                     tag="w")
                        nc.sync.dma_start(
                            wg[:],
                            wvT[1024 * gw:1024 * (gw + 1), :].rearrange(
                                "(k p) t -> p k t", p=128))
                        wv_s.extend(wg[:, i, :] for i in range(8))

                # ---- V projection (+ adapter V on half 0)
                for tb in range(S // 128):
                    ps = pps.tile([128, DSL], F32, name=f"psv{th}_{tb}",
                                  tag="proj")
                    for k in range(KD):
                        nc.tensor.matmul(
                            ps[:], x_strip[k][:, 128 * tb:128 * (tb + 1)],
                            wv_s[k][:], start=(k == 0), stop=(k == KD - 1))
                    nc.scalar.copy(Vt[th * (S // 128) + tb][:], ps[:])
                if th == 0:
                    for k in range(KD):
                        nc.tensor.matmul(av_ps[:AL, :], ad_strip[k][:],
                                         wv_s[k][:], start=(k == 0),
                                         stop=(k == KD - 1))
                    nc.scalar.copy(av_sb[:AL, :], av_ps[:AL, :])

                # ---- Q projection + RoPE
                wq_s = []
                for g in range(KD // 8):
                    wg = wp.tile([128, 8, DSL], BF, name=f"wq{th}_{g}",
                                 tag="w")
                    nc.sync.dma_start(
                        wg[:],
                        wqT[1024 * g:1024 * (g + 1), :].rearrange(
                            "(k p) t -> p k t", p=128))
                    wq_s.extend(wg[:, i, :] for i in range(8))
                for h in range(HPC):
                    for nb in range(S // 512):
                        scol = 512 * nb
                        ps = pps.tile([128, 512], F32, name=f"psq{th}{h}{nb}",
                                      tag="proj")
                        for k in range(KD):
                            nc.tensor.matmul(
                                ps[:], wq_s[k][:, 128 * h:128 * (h + 1)],
                                x_strip[k][:, scol:scol + 512],
                                start=(k == 0), stop=(k == KD - 1))
                        rope_evac(ps[:], QT[h][:, t0 + scol:t0 + scol + 512],
                                  scol)

                # ---- K projection + RoPE (+ adapter K on half 0)
                wk_s = []
                for g in range(KD // 8):
                    wg = wp.tile([128, 8, DSL], BF, name=f"wk{th}_{g}",
                                 tag="w")
                    nc.sync.dma_start(
                        wg[:],
                        wkT[1024 * g:1024 * (g + 1), :].rearrange(
                            "(k p) t -> p k t", p=128))
                    wk_s.extend(wg[:, i, :] for i in range(8))
                for h in range(HPC):
                    for nb in range(S // 512):
                        scol = 512 * nb
                        ps = pps.tile([128, 512], F32, name=f"psk{th}{h}{nb}",
                                      tag="proj")
                        for k in range(KD):
                            nc.tensor.matmul(
                                ps[:], wk_s[k][:, 128 * h:128 * (h + 1)],
                                x_strip[k][:, scol:scol + 512],
                                start=(k == 0), stop=(k == KD - 1))
                        rope_evac(ps[:], KT[h][:, t0 + scol:t0 + scol + 512],
                                  scol)
                if th == 0:
                    for h in range(HPC):
                        for k in range(KD):
                            nc.tensor.matmul(
                                ak_ps[:, AL * h:AL * (h + 1)],
                                wk_s[k][:, 128 * h:128 * (h + 1)],
                                ad_strip[k][:], start=(k == 0),
                                stop=(k == KD - 1))
                    nc.scalar.copy(akT_sb[:], ak_ps[:])

        def emit_attention_chunk(b, qc, pools):
            """Attention for 512 queries (all 4 heads) + its AllGather."""
            sps, ops, rsps, ptp, epp, acp = pools
            ch = 2 * b + qc
            tb0 = b * S
            q0 = qc * 512
            tq = tb0 + q0
            ktmax = 4 * qc + 3
            for h in range(HPC):
                # ---- transposed scores + exp, per key tile
                pts = []
                for kt in range(ktmax + 1):
                    lo = max(0, 128 * kt - q0)
                    s_ps = sps.tile([128, 512], F32,
                                    name=f"s{b}{h}{qc}{kt}", tag="s")
                    nc.tensor.matmul(
                        s_ps[:, lo:512],
                        KT[h][:, tb0 + 128 * kt:tb0 + 128 * (kt + 1)],
                        QT[h][:, tq + lo:tq + 512],
                        start=True, stop=True)
                    if kt >= 4 * qc:
                        # diagonal block: accumulate the mask on the PE
                        # (identity x mask = mask)
                        nc.tensor.matmul(
                            s_ps[:, lo:lo + 128], ident_bf[:],
                            md_sb[:, 128 * kt:128 * (kt + 1)],
                            start=False, stop=True, skip_group_check=True)
                    p_sb = ptp.tile([128, 512], BF,
                                    name=f"p{b}{h}{qc}{kt}", tag="pt")
                    nc.scalar.activation(p_sb[:, lo:512], s_ps[:, lo:512],
                                         EXP, scale=SCALE)
                    pts.append((kt, lo, p_sb))
                # ---- adapter scores + exp
                sa_ps = sps.tile([128, 512], F32, name=f"sa{b}{h}{qc}",
                                 tag="s")
                nc.tensor.matmul(sa_ps[:AL, :],
                                 akT_sb[:, AL * h:AL * (h + 1)],
                                 QT[h][:, tq:tq + 512], start=True, stop=True)
                pa_sb = ptp.tile([128, 512], BF, name=f"pa{b}{h}{qc}",
                                 tag="pt")
                nc.scalar.activation(pa_sb[:AL, :], sa_ps[:AL, :], EXP,
                                     scale=SCALE)
                # ---- PV + broadcast rowsums (kt-outer, column suffixes)
                o_m = ops.tile([128, 512], F32, name=f"om{b}{h}{qc}", tag="o")
                o_a = ops.tile([128, 512], F32, name=f"oa{b}{h}{qc}", tag="o")
                rs_m = rsps.tile([128, 512], F32, name=f"rm{b}{h}{qc}",
                                 tag="rs")
                rs_a = rsps.tile([128, 512], F32, name=f"ra{b}{h}{qc}",
                                 tag="rs")
                for kt, lo, p_sb in pts:
                    nc.tensor.matmul(
                        o_m[:, lo:512],
                        Vt[(S // 128) * b + kt][:, 128 * h:128 * (h + 1)],
                        p_sb[:, lo:512], start=(kt == 0), stop=True,
                        skip_group_check=(kt != 0))
                for kt, lo, p_sb in pts:
                    nc.tensor.matmul(
                        rs_m[:, lo:512], ones_bc[:], p_sb[:, lo:512],
                        start=(kt == 0), stop=True,
                        skip_group_check=(kt != 0))
                nc.tensor.matmul(o_a[:], av_sb[:AL, 128 * h:128 * (h + 1)],
                                 pa_sb[:AL, :], start=True, stop=True)
                nc.tensor.matmul(rs_a[:], ones_bc[:AL, :], pa_sb[:AL, :],
                                 start=True, stop=True)
                # ---- epilogue: normalize, gate, combine
                rec_m = epp.tile([128, 512], F32, name=f"cm{b}{h}{qc}",
                                 tag="rec")
                nc.vector.reciprocal_approx_fast(rec_m[:], rs_m[:])
                rec_a = epp.tile([128, 512], F32, name=f"ca{b}{h}{qc}",
                                 tag="reca")
                nc.vector.reciprocal_approx_fast(rec_a[:], rs_a[:])
                t1 = epp.tile([128, 512], BF, name=f"t1{b}{h}{qc}", tag="t1")
                nc.vector.tensor_mul(t1[:], o_m[:], rec_m[:])
                t2 = epp.tile([128, 512], BF, name=f"t2{b}{h}{qc}", tag="t2")
                nc.vector.scalar_tensor_tensor(t2[:], rec_a[:],
                                               tg128[:, h:h + 1], o_a[:],
                                               op0=MUL, op1=MUL)
                ac = acp.tile([128, 512], BF, name=f"ac{b}{h}{qc}", tag="ac")
                nc.vector.tensor_add(ac[:], t1[:], t2[:])
                # gpsimd queue: keeps the collective feed off the bulk
                # queues; gather this head right away
                nc.gpsimd.dma_start(ag_in[ch][h][:, :], ac[:])
                nc.gpsimd.collective_compute(
                    "AllGather", mybir.AluOpType.bypass,
                    replica_groups=[list(range(NCORES))],
                    ins=[ag_in[ch][h][:].opt()],
                    outs=[ag_out[ch][h][:].opt()],
                )

        def emit_wo_strips(ch, wox):
            """Prefetch the gathered chunk into SBUF (sync queue).

            Returns strips indexed by global k = 4*core + head (matching the
            woT row order)."""
            per_h = []
            for h in range(HPC):
                agt = wox.tile([128, 8, 512], BF, name=f"ag{ch}_{h}",
                               tag="ag")
                nc.sync.dma_start(
                    agt[:],
                    ag_out[ch][h].rearrange("(c p) t -> p c t", p=128))
                per_h.append(agt)
            return [per_h[k % HPC][:, k // HPC, :] for k in range(KD)]

        def emit_wo_chunk(ch, pools, ag_sb):
            """Wo projection for one gathered 512-token chunk."""
            _, wops, woo, wo_s = pools
            tq = 512 * ch
            for m in range(HPC):
                wps = wops.tile([128, 512], F32, name=f"wo{ch}{m}", tag="wo")
                for k in range(KD):
                    nc.tensor.matmul(wps[:],
                                     wo_s[k][:, 128 * m:128 * (m + 1)],
                                     ag_sb[k][:], start=(k == 0),
                                     stop=(k == KD - 1))
                o_sb = woo.tile([128, 512], F32, name=f"os{ch}{m}", tag="os")
                nc.vector.tensor_copy(o_sb[:], wps[:])
                nc.sync.dma_start(
                    out[128 * m:128 * (m + 1), tq:tq + 512], o_sb[:])

        def attention_pools(suffix, es, s_bufs=2):
            return (
                es.enter_context(tc.tile_pool(name=f"sps{suffix}",
                                              bufs=s_bufs, space="PSUM")),
                es.enter_context(tc.tile_pool(name=f"ops{suffix}", bufs=2,
                                              space="PSUM")),
                es.enter_context(tc.tile_pool(name=f"rsps{suffix}", bufs=2,
                                              space="PSUM")),
                es.enter_context(tc.tile_pool(name=f"ptp{suffix}", bufs=12)),
                es.enter_context(tc.tile_pool(name=f"epp{suffix}", bufs=3)),
                es.enter_context(tc.tile_pool(name=f"acp{suffix}", bufs=6)),
            )

        # ================= emission schedule ==============================
        emit_proj_half(0)

        with ExitStack() as es2:
            pools_a = attention_pools("a", es2, s_bufs=3)
            emit_attention_chunk(0, 0, pools_a)   # + AG0
            emit_attention_chunk(0, 1, pools_a)   # + AG1

        emit_proj_half(1)

        with ExitStack() as es4:
            pools_b = attention_pools("b", es4)
            wox = es4.enter_context(tc.tile_pool(name="wox", bufs=8))
            wops = es4.enter_context(tc.tile_pool(name="wops", bufs=2,
                                                  space="PSUM"))
            woo = es4.enter_context(tc.tile_pool(name="woo", bufs=4))
            wow = es4.enter_context(tc.tile_pool(name="wow", bufs=1))
            wo_s = []
            for g in range(KD // 8):
                wg = wow.tile([128, 8, DSL], BF, name=f"wo_g{g}")
                nc.sync.dma_start(
                    wg[:],
                    woT[1024 * g:1024 * (g + 1), :].rearrange(
                        "(k p) t -> p k t", p=128))
                wo_s.extend(wg[:, i, :] for i in range(8))
            wo_pools = (wox, wops, woo, wo_s)

            # ladder: every AllGather hides behind a Wo chunk or attention
            strips0 = emit_wo_strips(0, wox)
            strips1 = emit_wo_strips(1, wox)
            emit_attention_chunk(1, 0, pools_b)   # + AG2
            emit_wo_chunk(0, wo_pools, strips0)
            emit_attention_chunk(1, 1, pools_b)   # + AG3
            strips2 = emit_wo_strips(2, wox)
            emit_wo_chunk(1, wo_pools, strips1)
            strips3 = emit_wo_strips(3, wox)
            emit_wo_chunk(2, wo_pools, strips2)
            emit_wo_chunk(3, wo_pools, strips3)

    nc.finalize()
    return nc


def _get_nc():
    global _nc_cache
    if _nc_cache is None:
        _nc_cache = _build_nc()
    return _nc_cache


# --------------------------------------------------------------------- host
_EVEN_ODD = np.concatenate([np.arange(0, HD, 2), np.arange(1, HD, 2)])


def _perm_rows(w_slice):
    """Permute each head's 128 rows to [evens; odds] order."""
    w = w_slice.reshape(HPC, HD, D)
    return w[:, _EVEN_ODD, :].reshape(HPC * HD, D)


def prepare_in_maps(inputs):
    x = np.asarray(inputs["x"], np.float32)
    Wq = np.asarray(inputs["Wq"], np.float32)
    Wk = np.asarray(inputs["Wk"], np.float32)
    Wv = np.asarray(inputs["Wv"], np.float32)
    Wo = np.asarray(inputs["Wo"], np.float32)
    gate = np.asarray(inputs["gate"], np.float32).reshape(H)
    adapter = np.asarray(inputs["adapter"], np.float32).reshape(AL, D)
    fcos = np.asarray(inputs["freqs_cos"], np.float32)   # [S, HD/2]
    fsin = np.asarray(inputs["freqs_sin"], np.float32)
    mask = np.asarray(inputs["mask"], np.float32).reshape(S, S)

    # replicated operands
    xT = np.ascontiguousarray(x.reshape(T, D).T).astype(BF16)
    adT = np.ascontiguousarray(adapter.T).astype(BF16)
    # RoPE tables for the [evens; odds] permuted row layout
    fcos128 = np.concatenate([fcos.T, fcos.T], axis=0).astype(BF16)
    fsin128 = np.concatenate([-fsin.T, fsin.T], axis=0).astype(BF16)
    # transposed diagonal mask blocks, packed [128, 8*128]
    mdT = np.concatenate(
        [mask[128 * kt:128 * (kt + 1), 128 * kt:128 * (kt + 1)].T
         for kt in range(NKT)], axis=1).astype(BF16)
    mdT = np.ascontiguousarray(mdT)

    in_maps = []
    for c in range(NCORES):
        sl = slice(c * DSL, (c + 1) * DSL)
        wq_c = _perm_rows(Wq[sl])
        wk_c = _perm_rows(Wk[sl])
        in_maps.append({
            "xT": xT,
            "wqT": np.ascontiguousarray(wq_c.T).astype(BF16),
            "wkT": np.ascontiguousarray(wk_c.T).astype(BF16),
            "wvT": np.ascontiguousarray(Wv[sl].T).astype(BF16),
            "woT": np.ascontiguousarray(Wo[sl].T).astype(BF16),
            "adT": adT,
            "mdT": mdT,
            "fcos": fcos128,
            "fsin": fsin128,
            "gate": gate[c * HPC:(c + 1) * HPC].reshape(1, HPC).copy(),
            "ident": np.eye(128, dtype=np.float32).astype(BF16),
        })
    return in_maps


def assemble_output(results):
    full_T = np.concatenate([results[c]["out"] for c in range(NCORES)],
                            axis=0)          # [D, T]
    return np.ascontiguousarray(full_T.T).reshape(B, S, D).astype(np.float32)


def kernel(**inputs):
    from concourse.bass_utils import run_bass_kernel_spmd

    in_maps = prepare_in_maps(inputs)
    nc = _get_nc()
    res = run_bass_kernel_spmd(nc, in_maps, core_ids=list(range(NCORES)))
    return assemble_output(res.results)


# revision 49
# speedup vs baseline: 1.0814x; 1.0814x over previous
"""Trainium2 Bass kernel for nn_Attention_40510131536197.

RoPE attention layer with gated adapter (Llama-adapter style), B=2, S=1024,
D=4096, H=32 heads, head_dim=128, adapter_len=64.

Distribution: tensor-parallel over heads across 8 NeuronCores. Each core owns
4 heads (a 512-row slice of Wq/Wk/Wv, its gate slice and adapter-KV slice)
plus a 512-column slice of Wo. The attention output (feature-major layout
[512, 2048] per core) is AllGather'd across cores in four 512-token chunks,
then each core computes its 512-feature output slice of the Wo projection.
The host concatenates + transposes the 8 slices into the full
[2, 1024, 4096] output.

Phase interleaving hides every collective behind TensorEngine work:
  proj(batch0) -> attention(batch0) + AG0/AG1 -> proj(batch1)
  -> [Wo(ch0, ch1) | attention(batch1) | AG2/AG3] -> Wo(ch2, ch3)

All matmuls run in bf16 (f32 PSUM accumulation). Layout choices keep the
contraction dim on SBUF partitions for every matmul so the kernel needs no
on-device transposes:
  - projections computed as out.T = W.T-shard (lhsT) x x.T (rhs)
  - scores computed transposed: S_T[k, q]; softmax denominators via
    ones-broadcast matmuls; the additive mask is folded in on the PE via an
    identity-matrix matmul on the diagonal 128x128 blocks only (causal
    structure skips sub-diagonal blocks entirely)
  - PV uses V tiles (token-major) as lhsT and S_T as rhs, accumulating
    kt-outer with column-suffix ranges, so the attention output lands
    directly in the feature-major layout the Wo matmul wants.
RoPE runs on Vector/Scalar engines in the feature-major layout using a
host-side even/odd row permutation of Wq/Wk (invariant under the q.k
contraction).
"""

import math
import numpy as np
import ml_dtypes

# ---------------------------------------------------------------- constants
B, S, D, H = 2, 1024, 4096, 32
HD = D // H            # 128 head dim
AL = 64                # adapter length
NCORES = 8
HPC = H // NCORES      # 4 heads per core
DSL = HPC * HD         # 512 per-core feature slice
T = B * S              # 2048 tokens
SCALE = 1.0 / math.sqrt(HD)
NKT = S // 128         # 8 key tiles per batch
NCH = 4                # AllGather chunks (512 tokens each)

BF16 = ml_dtypes.bfloat16

_nc_cache = None


# ------------------------------------------------------------------- device
def _build_nc():
    import concourse.bacc as bacc
    import concourse.tile as tile
    from concourse import mybir
    from contextlib import ExitStack

    F32 = mybir.dt.float32
    BF = mybir.dt.bfloat16
    EXP = mybir.ActivationFunctionType.Exp
    TANH = mybir.ActivationFunctionType.Tanh
    MUL = mybir.AluOpType.mult

    nc = bacc.Bacc("TRN2", target_bir_lowering=False, debug=False,
                   num_devices=NCORES)

    # ---- external I/O (per-core shards; host prepares layouts/dtypes)
    xT = nc.dram_tensor("xT", [D, T], BF, kind="ExternalInput")
    wqT = nc.dram_tensor("wqT", [D, DSL], BF, kind="ExternalInput")
    wkT = nc.dram_tensor("wkT", [D, DSL], BF, kind="ExternalInput")
    wvT = nc.dram_tensor("wvT", [D, DSL], BF, kind="ExternalInput")
    woT = nc.dram_tensor("woT", [D, DSL], BF, kind="ExternalInput")
    adT = nc.dram_tensor("adT", [D, AL], BF, kind="ExternalInput")
    mdT = nc.dram_tensor("mdT", [128, NKT * 128], BF, kind="ExternalInput")
    fcos = nc.dram_tensor("fcos", [128, S], BF, kind="ExternalInput")
    fsin = nc.dram_tensor("fsin", [128, S], BF, kind="ExternalInput")
    gate = nc.dram_tensor("gate", [1, HPC], F32, kind="ExternalInput")
    ident = nc.dram_tensor("ident", [128, 128], BF, kind="ExternalInput")
    out = nc.dram_tensor("out", [DSL, T], F32, kind="ExternalOutput")

    KD = D // 128  # 32 contraction strips

    with tile.TileContext(nc) as tc, ExitStack() as stack:
        # ---------------- constants / persistent SBUF
        const = stack.enter_context(tc.tile_pool(name="const", bufs=1))
        qkv_sb = stack.enter_context(tc.tile_pool(name="qkv_sb", bufs=1))
        misc_sb = stack.enter_context(tc.tile_pool(name="misc_sb", bufs=1))
        dram = stack.enter_context(tc.tile_pool(name="dram", bufs=1,
                                                space="DRAM"))

        fcos_sb = const.tile([128, S], BF, name="fcos_sb")
        nc.sync.dma_start(fcos_sb[:], fcos[:, :])
        fsin_sb = const.tile([128, S], BF, name="fsin_sb")
        nc.sync.dma_start(fsin_sb[:], fsin[:, :])
        md_sb = const.tile([128, NKT * 128], BF, name="md_sb")
        nc.sync.dma_start(md_sb[:], mdT[:, :])
        ones_bc = const.tile([128, 128], BF, name="ones_bc")
        nc.vector.memset(ones_bc[:], 1.0)
        ident_bf = const.tile([128, 128], BF, name="ident_bf")
        nc.sync.dma_start(ident_bf[:], ident[:, :])
        # tanh(gate), broadcast to all partitions via a DRAM bounce
        g_raw = const.tile([1, HPC], F32, name="g_raw")
        nc.sync.dma_start(g_raw[:], gate[:, :])
        tg_row = const.tile([1, HPC], F32, name="tg_row")
        nc.scalar.activation(tg_row[:], g_raw[:], TANH)
        tg_d = dram.tile([1, HPC], F32, name="tg_d")
        nc.sync.dma_start(tg_d[:], tg_row[:])
        tg128 = const.tile([128, HPC], F32, name="tg128")
        nc.sync.dma_start(tg128[:], tg_d[0:1, :].to_broadcast((128, HPC)))

        # persistent activations
        QT = [qkv_sb.tile([128, T], BF, name=f"QT{h}") for h in range(HPC)]
        KT = [qkv_sb.tile([128, T], BF, name=f"KT{h}") for h in range(HPC)]
        Vt = [qkv_sb.tile([128, DSL], BF, name=f"V{i}")
              for i in range(T // 128)]
        akT_sb = misc_sb.tile([128, HPC * AL], BF, name="akT_sb")
        av_sb = misc_sb.tile([128, DSL], BF, name="av_sb")  # use [:AL]

        # collective bounce buffers, one per 512-token chunk
        ag_in = [dram.tile([DSL, 512], BF, name=f"ag_in{ch}")
                 for ch in range(NCH)]
        ag_out = [dram.tile([NCORES * DSL, 512], BF, addr_space="Shared",
                            name=f"ag_out{ch}") for ch in range(NCH)]

        # ---------------------------------------------------------- helpers
        def emit_proj_half(th):
            """Projections (V, Q+RoPE, K+RoPE, adapter KV on half 0) for one
            1024-token half (== batch)."""
            t0 = th * S
            with tc.tile_pool(name=f"xp{th}", bufs=8) as xp, \
                 tc.tile_pool(name=f"wp{th}", bufs=4) as wp, \
                 tc.tile_pool(name=f"pps{th}", bufs=4, space="PSUM") as pps, \
                 tc.tile_pool(name=f"adps{th}", bufs=1, space="PSUM") as adps, \
                 tc.tile_pool(name=f"rp{th}", bufs=3) as rp:

                def rope_evac(ps, dst_ap, scol):
                    """RoPE: out = v*C + rot(v)*S' (rows are [evens; odds]).
                    The rotation's half-swap runs as two 1-input ACT copies
                    (2-input SBUF ops must share base partitions)."""
                    vbf = rp.tile([128, 512], BF, name="vbf", tag="rv")
                    nc.vector.tensor_copy(vbf[:], ps[:])
                    vrot = rp.tile([128, 512], BF, name="vrot", tag="rr")
                    nc.vector.tensor_copy(vrot[0:64, :], ps[64:128, :])
                    nc.vector.tensor_copy(vrot[64:128, :], ps[0:64, :])
                    tcv = rp.tile([128, 512], BF, name="tcv", tag="rc")
                    nc.vector.tensor_mul(tcv[:], vbf[:],
                                         fcos_sb[:, scol:scol + 512])
                    tsv = rp.tile([128, 512], BF, name="tsv", tag="rs")
                    nc.vector.tensor_mul(tsv[:], vrot[:],
                                         fsin_sb[:, scol:scol + 512])
                    nc.vector.tensor_add(dst_ap, tcv[:], tsv[:])

                if th == 0:
                    ad_g = misc_sb.tile([128, KD, AL], BF, name="ad_g")
                    nc.sync.dma_start(
                        ad_g[:], adT.rearrange("(k p) a -> p k a", p=128))
                    ad_strip = [ad_g[:, k, :] for k in range(KD)]
                    ak_ps = adps.tile([128, HPC * AL], F32, name="ak_ps",
                                      tag="adk")
                    av_ps = adps.tile([128, DSL], F32, name="av_ps",
                                      tag="adv")

                # interleave x / wv group loads so matmuls start early
                x_strip = []
                wv_s = []
                for g in range(KD // 4):
                    xg = xp.tile([128, 4, S], BF, name=f"x{th}_{g}",
                                 tag="xstrip")
                    nc.sync.dma_start(
                        xg[:],
                        xT[512 * g:512 * (g + 1),
                           t0:t0 + S].rearrange("(k p) t -> p k t", p=128))
                    x_strip.extend(xg[:, i, :] for i in range(4))
                    if g % 2 == 0:
                        gw = g // 2
                        wg = wp.tile([128, 8, DSL], BF, name=f"wv{th}_{gw}",
                                     tag="w")
                        nc.sync.dma_start(
                            wg[:],
                            wvT[1024 * gw:1024 * (gw + 1), :].rearrange(
                                "(k p) t -> p k t", p=128))
                        wv_s.extend(wg[:, i, :] for i in range(8))

                # ---- V projection (+ adapter V on half 0)
                for tb in range(S // 128):
                    ps = pps.tile([128, DSL], F32, name=f"psv{th}_{tb}",
                                  tag="proj")
                    for k in range(KD):
                        nc.tensor.matmul(
                            ps[:], x_strip[k][:, 128 * tb:128 * (tb + 1)],
                            wv_s[k][:], start=(k == 0), stop=(k == KD - 1))
                    nc.scalar.copy(Vt[th * (S // 128) + tb][:], ps[:])
                if th == 0:
                    for k in range(KD):
                        nc.tensor.matmul(av_ps[:AL, :], ad_strip[k][:],
                                         wv_s[k][:], start=(k == 0),
                                         stop=(k == KD - 1))
                    nc.scalar.copy(av_sb[:AL, :], av_ps[:AL, :])

                # ---- Q projection + RoPE
                wq_s = []
                for g in range(KD // 8):
                    wg = wp.tile([128, 8, DSL], BF, name=f"wq{th}_{g}",
                                 tag="w")
                    nc.sync.dma_start(
                        wg[:],
                        wqT[1024 * g:1024 * (g + 1), :].rearrange(
                            "(k p) t -> p k t", p=128))
                    wq_s.extend(wg[:, i, :] for i in range(8))
                for h in range(HPC):
                    for nb in range(S // 512):
                        scol = 512 * nb
                        ps = pps.tile([128, 512], F32, name=f"psq{th}{h}{nb}",
                                      tag="proj")
                        for k in range(KD):
                            nc.tensor.matmul(
                                ps[:], wq_s[k][:, 128 * h:128 * (h + 1)],
                                x_strip[k][:, scol:scol + 512],
                                start=(k == 0), stop=(k == KD - 1))
                        rope_evac(ps[:], QT[h][:, t0 + scol:t0 + scol + 512],
                                  scol)

                # ---- K projection + RoPE (+ adapter K on half 0)
                wk_s = []
                for g in range(KD // 8):
                    wg = wp.tile([128, 8, DSL], BF, name=f"wk{th}_{g}",
                                 tag="w")
                    nc.sync.dma_start(
                        wg[:],
                        wkT[1024 * g:1024 * (g + 1), :].rearrange(
                            "(k p) t -> p k t", p=128))
                    wk_s.extend(wg[:, i, :] for i in range(8))
                for h in range(HPC):
                    for nb in range(S // 512):
                        scol = 512 * nb
                        ps = pps.tile([128, 512], F32, name=f"psk{th}{h}{nb}",
                                      tag="proj")
                        for k in range(KD):
                            nc.tensor.matmul(
                                ps[:], wk_s[k][:, 128 * h:128 * (h + 1)],
                                x_strip[k][:, scol:scol + 512],
                                start=(k == 0), stop=(k == KD - 1))
                        rope_evac(ps[:], KT[h][:, t0 + scol:t0 + scol + 512],
                                  scol)
                if th == 0:
                    for h in range(HPC):
                        for k in range(KD):
                            nc.tensor.matmul(
                                ak_ps[:, AL * h:AL * (h + 1)],
                                wk_s[k][:, 128 * h:128 * (h + 1)],
                                ad_strip[k][:], start=(k == 0),
                                stop=(k == KD - 1))
                    nc.scalar.copy(akT_sb[:], ak_ps[:])

        def emit_attention_chunk(b, qc, pools):
            """Attention for 512 queries (all 4 heads) + its AllGather."""
            sps, ops, rsps, ptp, epp, acp = pools
            ch = 2 * b + qc
            tb0 = b * S
            q0 = qc * 512
            tq = tb0 + q0
            ktmax = 4 * qc + 3
            for h in range(HPC):
                # ---- transposed scores + exp, per key tile
                pts = []
                for kt in range(ktmax + 1):
                    lo = max(0, 128 * kt - q0)
                    s_ps = sps.tile([128, 512], F32,
                                    name=f"s{b}{h}{qc}{kt}", tag="s")
                    nc.tensor.matmul(
                        s_ps[:, lo:512],
                        KT[h][:, tb0 + 128 * kt:tb0 + 128 * (kt + 1)],
                        QT[h][:, tq + lo:tq + 512],
                        start=True, stop=True)
                    if kt >= 4 * qc:
                        # diagonal block: accumulate the mask on the PE
                        # (identity x mask = mask)
                        nc.tensor.matmul(
                            s_ps[:, lo:lo + 128], ident_bf[:],
                            md_sb[:, 128 * kt:128 * (kt + 1)],
                            start=False, stop=True, skip_group_check=True)
                    p_sb = ptp.tile([128, 512], BF,
                                    name=f"p{b}{h}{qc}{kt}", tag="pt")
                    nc.scalar.activation(p_sb[:, lo:512], s_ps[:, lo:512],
                                         EXP, scale=SCALE)
                    pts.append((kt, lo, p_sb))
                # ---- adapter scores + exp
                sa_ps = sps.tile([128, 512], F32, name=f"sa{b}{h}{qc}",
                                 tag="s")
                nc.tensor.matmul(sa_ps[:AL, :],
                                 akT_sb[:, AL * h:AL * (h + 1)],
                                 QT[h][:, tq:tq + 512], start=True, stop=True)
                pa_sb = ptp.tile([128, 512], BF, name=f"pa{b}{h}{qc}",
                                 tag="pt")
                nc.scalar.activation(pa_sb[:AL, :], sa_ps[:AL, :], EXP,
                                     scale=SCALE)
                # ---- PV + broadcast rowsums (kt-outer, column suffixes)
                o_m = ops.tile([128, 512], F32, name=f"om{b}{h}{qc}", tag="o")
                o_a = ops.tile([128, 512], F32, name=f"oa{b}{h}{qc}", tag="o")
                rs_m = rsps.tile([128, 512], F32, name=f"rm{b}{h}{qc}",
                                 tag="rs")
                rs_a = rsps.tile([128, 512], F32, name=f"ra{b}{h}{qc}",
                                 tag="rs")
                for kt, lo, p_sb in pts:
                    nc.tensor.matmul(
                        o_m[:, lo:512],
                        Vt[(S // 128) * b + kt][:, 128 * h:128 * (h + 1)],
                        p_sb[:, lo:512], start=(kt == 0), stop=True,
                        skip_group_check=(kt != 0))
                for kt, lo, p_sb in pts:
                    nc.tensor.matmul(
                        rs_m[:, lo:512], ones_bc[:], p_sb[:, lo:512],
                        start=(kt == 0), stop=True,
                        skip_group_check=(kt != 0))
                nc.tensor.matmul(o_a[:], av_sb[:AL, 128 * h:128 * (h + 1)],
                                 pa_sb[:AL, :], start=True, stop=True)
                nc.tensor.matmul(rs_a[:], ones_bc[:AL, :], pa_sb[:AL, :],
                                 start=True, stop=True)
                # ---- epilogue: normalize, gate, combine
                rec_m = epp.tile([128, 512], F32, name=f"cm{b}{h}{qc}",
                                 tag="rec")
                nc.vector.reciprocal_approx_fast(rec_m[:], rs_m[:])
                rec_a = epp.tile([128, 512], F32, name=f"ca{b}{h}{qc}",
                                 tag="reca")
                nc.vector.reciprocal_approx_fast(rec_a[:], rs_a[:])
                t1 = epp.tile([128, 512], BF, name=f"t1{b}{h}{qc}", tag="t1")
                nc.vector.tensor_mul(t1[:], o_m[:], rec_m[:])
                t2 = epp.tile([128, 512], BF, name=f"t2{b}{h}{qc}", tag="t2")
                nc.vector.scalar_tensor_tensor(t2[:], rec_a[:],
                                               tg128[:, h:h + 1], o_a[:],
                                               op0=MUL, op1=MUL)
                ac = acp.tile([128, 512], BF, name=f"ac{b}{h}{qc}", tag="ac")
                nc.vector.tensor_add(ac[:], t1[:], t2[:])
                # gpsimd queue: keeps the collective feed off the bulk queues
                nc.gpsimd.dma_start(ag_in[ch][128 * h:128 * (h + 1), :],
                                    ac[:])
            nc.gpsimd.collective_compute(
                "AllGather", mybir.AluOpType.bypass,
                replica_groups=[list(range(NCORES))],
                ins=[ag_in[ch][:].opt()],
                outs=[ag_out[ch][:].opt()],
            )

        def emit_wo_strips(ch, wox):
            """Prefetch the gathered chunk into SBUF (sync queue)."""
            ag_g = []
            for g in range(KD // 8):
                agt = wox.tile([128, 8, 512], BF, name=f"ag{ch}_{g}",
                               tag="ag")
                nc.sync.dma_start(
                    agt[:],
                    ag_out[ch][1024 * g:1024 * (g + 1), :].rearrange(
                        "(k p) t -> p k t", p=128))
                ag_g.append(agt)
            return [agt[:, i, :] for agt in ag_g for i in range(8)]

        def emit_wo_chunk(ch, pools, ag_sb):
            """Wo projection for one gathered 512-token chunk."""
            _, wops, woo, wo_s = pools
            tq = 512 * ch
            for m in range(HPC):
                wps = wops.tile([128, 512], F32, name=f"wo{ch}{m}", tag="wo")
                for k in range(KD):
                    nc.tensor.matmul(wps[:],
                                     wo_s[k][:, 128 * m:128 * (m + 1)],
                                     ag_sb[k][:], start=(k == 0),
                                     stop=(k == KD - 1))
                o_sb = woo.tile([128, 512], F32, name=f"os{ch}{m}", tag="os")
                nc.vector.tensor_copy(o_sb[:], wps[:])
                nc.sync.dma_start(
                    out[128 * m:128 * (m + 1), tq:tq + 512], o_sb[:])

        def attention_pools(suffix, es, s_bufs=2):
            return (
                es.enter_context(tc.tile_pool(name=f"sps{suffix}",
                                              bufs=s_bufs, space="PSUM")),
                es.enter_context(tc.tile_pool(name=f"ops{suffix}", bufs=2,
                                              space="PSUM")),
                es.enter_context(tc.tile_pool(name=f"rsps{suffix}", bufs=2,
                                              space="PSUM")),
                es.enter_context(tc.tile_pool(name=f"ptp{suffix}", bufs=12)),
                es.enter_context(tc.tile_pool(name=f"epp{suffix}", bufs=3)),
                es.enter_context(tc.tile_pool(name=f"acp{suffix}", bufs=6)),
            )

        # ================= emission schedule ==============================
        emit_proj_half(0)

        with ExitStack() as es2:
            pools_a = attention_pools("a", es2, s_bufs=3)
            emit_attention_chunk(0, 0, pools_a)   # + AG0
            emit_attention_chunk(0, 1, pools_a)   # + AG1

        emit_proj_half(1)

        with ExitStack() as es4:
            pools_b = attention_pools("b", es4)
            wox = es4.enter_context(tc.tile_pool(name="wox", bufs=8))
            wops = es4.enter_context(tc.tile_pool(name="wops", bufs=2,
                                                  space="PSUM"))
            woo = es4.enter_context(tc.tile_pool(name="woo", bufs=4))
            wow = es4.enter_context(tc.tile_pool(name="wow", bufs=1))
            wo_s = []
            for g in range(KD // 8):
                wg = wow.tile([128, 8, DSL], BF, name=f"wo_g{g}")
                nc.sync.dma_start(
                    wg[:],
                    woT[1024 * g:1024 * (g + 1), :].rearrange(
                        "(k p) t -> p k t", p=128))
                wo_s.extend(wg[:, i, :] for i in range(8))
            wo_pools = (wox, wops, woo, wo_s)

            # ladder: every AllGather hides behind a Wo chunk or attention
            strips0 = emit_wo_strips(0, wox)
            strips1 = emit_wo_strips(1, wox)
            emit_attention_chunk(1, 0, pools_b)   # + AG2
            emit_wo_chunk(0, wo_pools, strips0)
            strips2 = emit_wo_strips(2, wox)
            emit_attention_chunk(1, 1, pools_b)   # + AG3
            strips3 = emit_wo_strips(3, wox)
            emit_wo_chunk(1, wo_pools, strips1)
            emit_wo_chunk(2, wo_pools, strips2)
            emit_wo_chunk(3, wo_pools, strips3)

    nc.finalize()
    return nc


def _get_nc():
    global _nc_cache
    if _nc_cache is None:
        _nc_cache = _build_nc()
    return _nc_cache


# --------------------------------------------------------------------- host
_EVEN_ODD = np.concatenate([np.arange(0, HD, 2), np.arange(1, HD, 2)])


def _perm_rows(w_slice):
    """Permute each head's 128 rows to [evens; odds] order."""
    w = w_slice.reshape(HPC, HD, D)
    return w[:, _EVEN_ODD, :].reshape(HPC * HD, D)


def prepare_in_maps(inputs):
    x = np.asarray(inputs["x"], np.float32)
    Wq = np.asarray(inputs["Wq"], np.float32)
    Wk = np.asarray(inputs["Wk"], np.float32)
    Wv = np.asarray(inputs["Wv"], np.float32)
    Wo = np.asarray(inputs["Wo"], np.float32)
    gate = np.asarray(inputs["gate"], np.float32).reshape(H)
    adapter = np.asarray(inputs["adapter"], np.float32).reshape(AL, D)
    fcos = np.asarray(inputs["freqs_cos"], np.float32)   # [S, HD/2]
    fsin = np.asarray(inputs["freqs_sin"], np.float32)
    mask = np.asarray(inputs["mask"], np.float32).reshape(S, S)

    # replicated operands
    xT = np.ascontiguousarray(x.reshape(T, D).T).astype(BF16)
    adT = np.ascontiguousarray(adapter.T).astype(BF16)
    # RoPE tables for the [evens; odds] permuted row layout
    fcos128 = np.concatenate([fcos.T, fcos.T], axis=0).astype(BF16)
    fsin128 = np.concatenate([-fsin.T, fsin.T], axis=0).astype(BF16)
    # transposed diagonal mask blocks, packed [128, 8*128]
    mdT = np.concatenate(
        [mask[128 * kt:128 * (kt + 1), 128 * kt:128 * (kt + 1)].T
         for kt in range(NKT)], axis=1).astype(BF16)
    mdT = np.ascontiguousarray(mdT)

    in_maps = []
    for c in range(NCORES):
        sl = slice(c * DSL, (c + 1) * DSL)
        wq_c = _perm_rows(Wq[sl])
        wk_c = _perm_rows(Wk[sl])
        in_maps.append({
            "xT": xT,
            "wqT": np.ascontiguousarray(wq_c.T).astype(BF16),
            "wkT": np.ascontiguousarray(wk_c.T).astype(BF16),
            "wvT": np.ascontiguousarray(Wv[sl].T).astype(BF16),
            "woT": np.ascontiguousarray(Wo[sl].T).astype(BF16),
            "adT": adT,
            "mdT": mdT,
            "fcos": fcos128,
            "fsin": fsin128,
            "gate": gate[c * HPC:(c + 1) * HPC].reshape(1, HPC).copy(),
            "ident": np.eye(128, dtype=np.float32).astype(BF16),
        })
    return in_maps


def assemble_output(results):
    full_T = np.concatenate([results[c]["out"] for c in range(NCORES)],
                            axis=0)          # [D, T]
    return np.ascontiguousarray(full_T.T).reshape(B, S, D).astype(np.float32)


def kernel(**inputs):
    from concourse.bass_utils import run_bass_kernel_spmd

    in_maps = prepare_in_maps(inputs)
    nc = _get_nc()
    res = run_bass_kernel_spmd(nc, in_maps, core_ids=list(range(NCORES)))
    return assemble_output(res.results)
